# revision 5
# baseline (speedup 1.0000x reference)
"""Trainium2 Bass kernel: dense MoE (BastileGptOssExperts) via expert parallelism.

Reference math (per token t, hidden h):
    out[t,h] = sum_e rw[t,e] * ( geglu(x @ W1_e + b1_e) @ W2_e + b2_e )[t,h]
with geglu(gate_up): gate = gu[..., 0::2], up = gu[..., 1::2]
    gate_c = min(gate, 7); up_c = clip(up, -7, 7)
    act = (up_c + 1) * gate_c * sigmoid(1.702 * gate_c)

8 cores, expert e on core e. Each core computes its expert over all T
tokens, weights the result by rw[:, e]; a per-token-chunk ReduceScatter
sums over cores and leaves each core with interleaved T/8 token slices;
the host reassembles.

Phase 1: gate_upT[d,t] = sum_k W1[k,d]^T xT[k,t] (W1 resident bf16,
  xT chunked/double-buffered), geglu fused on DVE+ACT:
    DVE: tg = (psum_g + b1g) min 7         [per-partition bias, dual-op]
    ACT: glu' = Silu(1.702 * tg)           [= 1.702 * glu_ref, bf16]
    DVE: tu = (psum_u + b1u) min 7
    ACT: r = Relu(tu + 7)                  [= clip(u,±7) + 7, bf16]
    DVE: act' = (r - 6) * glu'             [= 1.702 * act_ref, bf16]
  act' spilled to DRAM [D, T] bf16. The 1/1.702 is folded into W2 on host.
Phase 2: psum[t,h] = sum_k actT[k,t]^T W2'[k,h]; DVE adds the broadcast
  b2 tile and scales by rw[t] per-partition, DMA'd to partial [T, H] f32.
Uniform per-256-token ReduceScatters (pipelined against phase-2 compute,
with the rs_out->out copy lagging one group) -> each core's out slices.

DMA semaphores are per-buffer-slot so every wait threshold is exact even
if DMA completions reorder across in-flight transfers.

build_nc(reps=N) emits the whole pipeline N times (same buffers; sem
thresholds continue) so on-device time can be measured as a slope.
"""
import sys
sys.path.insert(0, "/opt/trn_rl_repo")

import numpy as np
import ml_dtypes

import concourse.bass as bass
import concourse.mybir as mybir
from concourse.bass_utils import run_bass_kernel_spmd

ALPHA = 1.702
LIMIT = 7.0
BF16 = mybir.dt.bfloat16
F32 = mybir.dt.float32
N_CORES = 8


def _cnt(G, s, m):
    """#{i in [0, G) : i % m == s}"""
    return G // m + (1 if s < G % m else 0)


def build_nc(H, D, T, TCH=512, reps=1, dbg=False, trickle=True, h_inner=True):
    KT = H // 128
    DT = D // 128
    KT2 = D // 128
    CH = T // TCH
    TS = TCH // 128
    HW2 = min(512, H)       # phase-2 matmul moving width (walrus caps at 512)
    HT2 = H // HW2
    N1 = CH * DT
    N2 = CH * TS * HT2
    PC2 = TS * HT2          # phase-2 tiles per chunk
    TCOL = T // 128
    RSC = 256               # tokens per ReduceScatter (uniform size required!)
    RSN = T // RSC          # collectives per rep
    RPG = RSC // 128 * HT2  # phase-2 out tiles per RS group
    RSO = RSC // N_CORES    # rs_out rows per group
    CCR = RSN
    W2G = 4                 # w2 load k-groups (trickle)
    M2 = CH * TS if h_inner else N2          # mm2_sem increments per rep
    M2C = TS if h_inner else PC2             # ... per chunk

    nc = bass.Bass()

    xt_ext = nc.declare_dram_parameter("xt", [H, T], BF16, isOutput=False)
    w1g_ext = nc.declare_dram_parameter("w1g", [H, D], BF16, isOutput=False)
    w1u_ext = nc.declare_dram_parameter("w1u", [H, D], BF16, isOutput=False)
    w2_ext = nc.declare_dram_parameter("w2", [D, H], BF16, isOutput=False)
    b1g_ext = nc.declare_dram_parameter("b1g", [128, DT], F32, isOutput=False)
    b1u_ext = nc.declare_dram_parameter("b1u", [128, DT], F32, isOutput=False)
    b2bc_ext = nc.declare_dram_parameter("b2bc", [128, H], F32, isOutput=False)
    rw_ext = nc.declare_dram_parameter("rw", [128, TCOL], F32, isOutput=False)
    out_ext = nc.declare_dram_parameter("out", [T // N_CORES, H], BF16, isOutput=True)
    dbg_ext = nc.declare_dram_parameter("dbg", [T, H], BF16, isOutput=True) if dbg else None

    act_sp = nc.dram_tensor("act_sp", [D, T], BF16)
    partial = nc.dram_tensor("partial", [T, H], BF16)
    rs_out = nc.dram_tensor("rs_out", [T // N_CORES, H], BF16)
    RSCH = TCH // N_CORES   # rs_out rows per chunk

    from contextlib import ExitStack
    with ExitStack() as ctx:
        ec = ctx.enter_context
        b1g_sb = ec(nc.sbuf_tensor("b1g_sb", [128, DT], F32))
        b1u_sb = ec(nc.sbuf_tensor("b1u_sb", [128, DT], F32))
        rw_sb = ec(nc.sbuf_tensor("rw_sb", [128, TCOL], F32))
        lim_sb = ec(nc.sbuf_tensor("lim_sb", [128, 1], F32))
        tg_sb = ec(nc.sbuf_tensor("tg_sb", [128, 2 * 512], BF16))
        tu_sb = ec(nc.sbuf_tensor("tu_sb", [128, 2 * 512], BF16))
        glu_sb = ec(nc.sbuf_tensor("glu_sb", [128, 2 * 512], BF16))
        r_sb = ec(nc.sbuf_tensor("r_sb", [128, 2 * 512], F32))
        acto_sb = ec(nc.sbuf_tensor("acto_sb", [128, 3 * 512], BF16))
        # compute-side semaphores (single-engine in-order increments)
        mm_sem = ec(nc.semaphore("mm_sem"))
        mm2_sem = ec(nc.semaphore("mm2_sem"))
        psum_free_sem = ec(nc.semaphore("psum_free_sem"))
        v1_sem = ec(nc.semaphore("v1_sem"))
        a_sem = ec(nc.semaphore("a_sem"))
        act_ready_sem = ec(nc.semaphore("act_ready_sem"))
        dve2_sem = ec(nc.semaphore("dve2_sem"))
        cc_sem = ec(nc.semaphore("cc_sem"))
        fin_sem = ec(nc.semaphore("fin_sem"))
        # DMA semaphores: per buffer slot so thresholds are exact
        dma_c_sem = ec(nc.semaphore("dma_c_sem"))
        w_sem = ec(nc.semaphore("w_sem"))
        w2g_sems = [ec(nc.semaphore(f"w2g_sem{i}")) for i in range(W2G)]
        wg_sems = [ec(nc.semaphore(f"wg_sem{i}")) for i in range(4)]
        xt_sems = [ec(nc.semaphore(f"xt_sem{i}")) for i in range(4)]
        sp_sems = [ec(nc.semaphore(f"sp_sem{i}")) for i in range(3)]
        rel_sems = [ec(nc.semaphore(f"rel_sem{i}")) for i in range(2)]
        op_sems = [ec(nc.semaphore(f"op_sem{i}")) for i in range(3)]
        b2_sem = ec(nc.semaphore("b2_sem"))
        block = ec(nc.Block())

        def tg(i):
            return tg_sb[:, (i % 2) * 512:(i % 2) * 512 + 512]

        def tu(i):
            return tu_sb[:, (i % 2) * 512:(i % 2) * 512 + 512]

        def glu(i):
            return glu_sb[:, (i % 2) * 512:(i % 2) * 512 + 512]

        def rr(i):
            return r_sb[:, (i % 2) * 512:(i % 2) * 512 + 512]

        def acto(i):
            return acto_sb[:, (i % 3) * 512:(i % 3) * 512 + 512]

        outs_holder = {}

        def outs(i):
            return outs_holder["sb"][:, (i % 3) * HW2:(i % 3) * HW2 + HW2]

        for rep in range(reps):
            # ======================= PHASE 1 =======================
            with (
                nc.sbuf_tensor(f"w1g_sb{rep}", [128, KT * D], BF16) as w1g_sb,
                nc.sbuf_tensor(f"w1u_sb{rep}", [128, KT * D], BF16) as w1u_sb,
                nc.sbuf_tensor(f"xt_sb{rep}", [128, 4 * KT * TCH], BF16) as xt_sb,
                nc.psum_tensor(f"psg{rep}", [128, 4 * 512], F32) as psg,
                nc.psum_tensor(f"psu{rep}", [128, 4 * 512], F32) as psu,
            ):
                def xt_t(c, k):
                    s = (c % 4) * KT * TCH
                    return xt_sb[:, s + k * TCH:s + k * TCH + TCH]

                def pg(pidx):  # slot by (q parity, pair member)
                    s = (((pidx // 2) % 2) * 2 + (pidx % 2)) * 512
                    return psg[:, s:s + 512]

                def pu(pidx):
                    s = (((pidx // 2) % 2) * 2 + (pidx % 2)) * 512
                    return psu[:, s:s + 512]

                @block.sync
                def _(sp, rep=rep, xt_t=xt_t):
                    if rep == 0:
                        sp.dma_start(out=b1g_sb[:], in_=b1g_ext[:]).then_inc(dma_c_sem, 16)
                        sp.dma_start(out=b1u_sb[:], in_=b1u_ext[:]).then_inc(dma_c_sem, 16)
                        sp.dma_start(out=rw_sb[:], in_=rw_ext[:]).then_inc(dma_c_sem, 16)
                    else:
                        # w1/xt regions were reused by phase 2 of rep-1
                        sp.wait_ge(mm2_sem, rep * M2)
                    GK1 = max(1, KT // 4)
                    for c in range(min(4, CH)):
                        gc = rep * CH + c
                        if rep == 0 and c == 0:
                            # interleave w1 + xt0 + xt1 loads in k-groups so PE
                            # can start pair 0 after the first group lands
                            for k in range(KT):
                                g = k // GK1
                                sp.dma_start(out=w1g_sb[:, k * D:(k + 1) * D],
                                             in_=w1g_ext[k * 128:(k + 1) * 128, :]).then_inc(wg_sems[g], 16)
                                sp.dma_start(out=w1u_sb[:, k * D:(k + 1) * D],
                                             in_=w1u_ext[k * 128:(k + 1) * 128, :]).then_inc(wg_sems[g], 16)
                                sp.dma_start(out=xt_t(0, k),
                                             in_=xt_ext[k * 128:(k + 1) * 128, 0:TCH]).then_inc(wg_sems[g], 16)
                                if CH > 1:
                                    sp.dma_start(out=xt_t(1, k),
                                                 in_=xt_ext[k * 128:(k + 1) * 128,
                                                            TCH:2 * TCH]).then_inc(wg_sems[g], 16)
                            continue
                        if rep == 0 and c == 1:
                            continue  # loaded with pair 0 above
                        for k in range(KT):
                            sp.dma_start(out=xt_t(c, k),
                                         in_=xt_ext[k * 128:(k + 1) * 128,
                                                    c * TCH:(c + 1) * TCH]).then_inc(xt_sems[gc % 4], 16)
                        if c == 0:
                            for k in range(KT):
                                sp.dma_start(out=w1g_sb[:, k * D:(k + 1) * D],
                                             in_=w1g_ext[k * 128:(k + 1) * 128, :]).then_inc(w_sem, 16)
                                sp.dma_start(out=w1u_sb[:, k * D:(k + 1) * D],
                                             in_=w1u_ext[k * 128:(k + 1) * 128, :]).then_inc(w_sem, 16)
                    for pidx in range(N1):
                        gj = rep * N1 + pidx
                        q, which = divmod(pidx, 2)
                        p, d = divmod(q, DT)
                        c = 2 * p + which
                        sp.wait_ge(act_ready_sem, gj + 1)
                        sp.dma_start(out=act_sp[d * 128:(d + 1) * 128, c * TCH:(c + 1) * TCH],
                                     in_=acto(gj)).then_inc(sp_sems[gj % 3], 16)
                        if pidx == 2 * DT * (p + 1) - 1 and 2 * (p + 2) < CH:
                            sp.wait_ge(mm_sem, 2 * rep * N1 + 4 * DT * (p + 1))
                            for cc in (2 * (p + 2), 2 * (p + 2) + 1):
                                gc2 = rep * CH + cc
                                for k in range(KT):
                                    sp.dma_start(out=xt_t(cc, k),
                                                 in_=xt_ext[k * 128:(k + 1) * 128,
                                                            cc * TCH:(cc + 1) * TCH]).then_inc(xt_sems[gc2 % 4], 16)

                @block.tensor
                def _(pe, rep=rep, xt_t=xt_t, pg=pg, pu=pu):
                    GK1 = max(1, KT // 4)
                    NP = CH // 2          # chunk pairs
                    for q in range(NP * DT):
                        p, d = divmod(q, DT)
                        gq = rep * NP * DT + q
                        c0, c1 = 2 * p, 2 * p + 1
                        gc0, gc1 = rep * CH + c0, rep * CH + c1
                        pA = rep * N1 + 2 * q      # pidx of pair members
                        if d == 0 and gc0 > 0:
                            for gcx in (gc0, gc1):
                                n = (gcx - 2) // 4 + 1
                                pe.wait_ge(xt_sems[gcx % 4], 16 * KT * n)
                        if q == 0 and rep > 0:
                            pe.wait_ge(w_sem, 16 * 2 * KT * rep)
                            pe.wait_ge(dve2_sem, rep * N2)
                        if gq >= 2:
                            pe.wait_ge(psum_free_sem, 2 * gq - 2)
                        for k in range(KT):
                            if gc0 == 0 and k % GK1 == 0:
                                pe.wait_ge(wg_sems[k // GK1], 16 * 4 * GK1)
                            mmA = pe.matmul(pg(pA), w1g_sb[:, k * D + d * 128:k * D + d * 128 + 128],
                                            xt_t(c0, k), start=(k == 0), stop=(k == KT - 1))
                            mmB = pe.matmul(pg(pA + 1), w1g_sb[:, k * D + d * 128:k * D + d * 128 + 128],
                                            xt_t(c1, k), start=(k == 0), stop=(k == KT - 1))
                        mmA.then_inc(mm_sem, 1)
                        mmB.then_inc(mm_sem, 1)
                        for k in range(KT):
                            mmA = pe.matmul(pu(pA), w1u_sb[:, k * D + d * 128:k * D + d * 128 + 128],
                                            xt_t(c0, k), start=(k == 0), stop=(k == KT - 1))
                            mmB = pe.matmul(pu(pA + 1), w1u_sb[:, k * D + d * 128:k * D + d * 128 + 128],
                                            xt_t(c1, k), start=(k == 0), stop=(k == KT - 1))
                        mmA.then_inc(mm_sem, 1)
                        mmB.then_inc(mm_sem, 1)

                @block.vector
                def _(dve, rep=rep, pg=pg, pu=pu):
                    if rep == 0:
                        dve.memset(lim_sb[:], LIMIT)
                        dve.wait_ge(dma_c_sem, 48)
                    for idx in range(N1):
                        gj = rep * N1 + idx
                        if gj >= 2:
                            dve.wait_ge(a_sem, gj - 1)
                        q, which = divmod(idx, 2)
                        d = q % DT
                        mmbase = 2 * rep * N1 + 4 * q
                        dve.wait_ge(mm_sem, mmbase + 1 + which)
                        dve.tensor_scalar(tg(gj), pg(gj), b1g_sb[:, d:d + 1], LIMIT,
                                          mybir.AluOpType.add, mybir.AluOpType.min).then_inc(v1_sem, 1)
                        dve.wait_ge(mm_sem, mmbase + 3 + which)
                        dve.tensor_scalar(tu(gj), pu(gj), b1u_sb[:, d:d + 1], LIMIT,
                                          mybir.AluOpType.add, mybir.AluOpType.min).then_inc(psum_free_sem, 1)
                        j = gj - 1
                        if j >= rep * N1:
                            dve.wait_ge(a_sem, j + 1)
                            if j >= 3:
                                dve.wait_ge(sp_sems[j % 3], 16 * (j // 3))
                            dve.scalar_tensor_tensor(acto(j), rr(j), 6.0, glu(j),
                                                     mybir.AluOpType.subtract,
                                                     mybir.AluOpType.mult).then_inc(act_ready_sem, 1)
                    j = rep * N1 + N1 - 1
                    dve.wait_ge(a_sem, j + 1)
                    if j >= 3:
                        dve.wait_ge(sp_sems[j % 3], 16 * (j // 3))
                    dve.scalar_tensor_tensor(acto(j), rr(j), 6.0, glu(j),
                                             mybir.AluOpType.subtract,
                                             mybir.AluOpType.mult).then_inc(act_ready_sem, 1)

                @block.scalar
                def _(act, rep=rep):
                    for idx in range(N1):
                        gj = rep * N1 + idx
                        if gj >= 2:
                            act.wait_ge(act_ready_sem, gj - 1)
                        act.wait_ge(v1_sem, gj + 1)
                        act.activation(glu(gj), tg(gj),
                                       mybir.ActivationFunctionType.Silu, scale=ALPHA)
                        act.wait_ge(psum_free_sem, gj + 1)
                        act.activation(rr(gj), tu(gj),
                                       mybir.ActivationFunctionType.Relu,
                                       bias=lim_sb[:, 0:1]).then_inc(a_sem, 1)

            # ======================= PHASE 2 =======================
            with (
                nc.sbuf_tensor(f"w2_sb{rep}", [128, KT2 * H], BF16) as w2_sb,
                nc.sbuf_tensor(f"actre_sb{rep}", [128, 2 * KT2 * TCH], BF16) as actre_sb,
                nc.sbuf_tensor(f"b2bc_sb{rep}", [128, H], F32) as b2bc_sb,
                nc.sbuf_tensor(f"outs_sb{rep}", [128, 3 * HW2], BF16) as outs_sb,
                nc.psum_tensor(f"pso{rep}", [128, (8 if h_inner else 3) * HW2], F32) as pso,
            ):
                def are_t(c, k):
                    s = (c % 2) * KT2 * TCH
                    return actre_sb[:, s + k * TCH:s + k * TCH + TCH]

                def po(i):
                    return pso[:, (i % 3) * HW2:(i % 3) * HW2 + HW2]

                def po2(g, h):
                    s = ((g % 2) * HT2 + h) * HW2
                    return pso[:, s:s + HW2]

                outs_holder["sb"] = outs_sb

                @block.sync
                def _(sp, rep=rep, are_t=are_t, b2bc_sb=b2bc_sb):
                    sp.wait_ge(mm_sem, 2 * N1 * (rep + 1))   # w1/xt regions free
                    if rep > 0:
                        # outs/b2bc overlay region also held prev rep's out tiles
                        for s in range(3):
                            v = 16 * _cnt(rep * N2, s, 3)
                            if v > 0:
                                sp.wait_ge(op_sems[s], v)
                    sp.dma_start(out=b2bc_sb[:], in_=b2bc_ext[:]).then_inc(b2_sem, 16)
                    GK = KT2 // W2G
                    # reload chunk 0 first (needed as soon as w2 group 0 lands)
                    G = rep * N1 + 2 * DT
                    for s in range(3):
                        v = 16 * _cnt(G, s, 3)
                        if v > 0:
                            sp.wait_ge(sp_sems[s], v)
                    for k in range(KT2):
                        sp.dma_start(out=are_t(0, k),
                                     in_=act_sp[k * 128:(k + 1) * 128, 0:TCH]).then_inc(
                                         rel_sems[(rep * CH) % 2], 16)
                    for k in range(KT2):
                        sp.dma_start(out=w2_sb[:, k * H:(k + 1) * H],
                                     in_=w2_ext[k * 128:(k + 1) * 128, :]).then_inc(
                                         w2g_sems[k // GK], 16)
                    if CH > 1:
                        gc = rep * CH + 1
                        G = rep * N1 + 2 * DT
                        for s in range(3):
                            v = 16 * _cnt(G, s, 3)
                            if v > 0:
                                sp.wait_ge(sp_sems[s], v)
                        for k in range(KT2):
                            sp.dma_start(out=are_t(1, k),
                                         in_=act_sp[k * 128:(k + 1) * 128,
                                                    TCH:2 * TCH]).then_inc(rel_sems[gc % 2], 16)
                    for idx2 in range(N2):
                        gj2 = rep * N2 + idx2
                        c, r0 = divmod(idx2, PC2)
                        tsub, h = divmod(r0, HT2)
                        if rep > 0 and r0 == 0:
                            # partial chunk c is re-written; previous rep's RS
                            # groups covering it must have consumed it
                            sp.wait_ge(cc_sem, (rep - 1) * CCR + (c + 1) * TCH // RSC)
                        sp.wait_ge(dve2_sem, gj2 + 1)
                        sp.dma_start(out=partial[c * TCH + tsub * 128:c * TCH + tsub * 128 + 128,
                                                 h * HW2:(h + 1) * HW2],
                                     in_=outs(gj2)).then_inc(op_sems[gj2 % 3], 16)
                        if r0 == PC2 - 1 and (c + 2) < CH:
                            gc2 = rep * CH + c + 2
                            sp.wait_ge(mm2_sem, rep * M2 + M2C * (c + 1))
                            G = rep * N1 + 2 * DT * ((c + 2) // 2 + 1)
                            for s in range(3):
                                v = 16 * _cnt(G, s, 3)
                                if v > 0:
                                    sp.wait_ge(sp_sems[s], v)
                            for k in range(KT2):
                                sp.dma_start(out=are_t(c + 2, k),
                                             in_=act_sp[k * 128:(k + 1) * 128,
                                                        (c + 2) * TCH:(c + 3) * TCH]).then_inc(rel_sems[gc2 % 2], 16)

                @block.tensor
                def _(pe, rep=rep, are_t=are_t, po=po, po2=po2):
                    GK = KT2 // W2G
                    if h_inner:
                        for g in range(CH * TS):
                            c, tsub = divmod(g, TS)
                            gg = rep * CH * TS + g
                            gc = rep * CH + c
                            if g == 0:
                                pe.wait_ge(psum_free_sem, N1 * (rep + 1))
                            if tsub == 0:
                                pe.wait_ge(rel_sems[gc % 2], 16 * KT2 * (gc // 2 + 1))
                            if gg >= 2:
                                # bank-set gg%2 reused from group gg-2: its
                                # HT2 tiles must be consumed by DVE4
                                pe.wait_ge(dve2_sem, HT2 * (gg - 1))
                            for k in range(KT2):
                                if g == 0 and k % GK == 0:
                                    pe.wait_ge(w2g_sems[k // GK], 16 * GK * (rep + 1))
                                for h in range(HT2):
                                    mm = pe.matmul(po2(gg, h),
                                                   are_t(c, k)[:, tsub * 128:tsub * 128 + 128],
                                                   w2_sb[:, k * H + h * HW2:k * H + h * HW2 + HW2],
                                                   start=(k == 0), stop=(k == KT2 - 1))
                            mm.then_inc(mm2_sem, 1)
                        return
                    for idx2 in range(N2):
                        gj2 = rep * N2 + idx2
                        c, r0 = divmod(idx2, PC2)
                        tsub, h = divmod(r0, HT2)
                        gc = rep * CH + c
                        GK = KT2 // W2G
                        if idx2 == 0:
                            pe.wait_ge(psum_free_sem, N1 * (rep + 1))
                        if r0 == 0:
                            pe.wait_ge(rel_sems[gc % 2], 16 * KT2 * (gc // 2 + 1))
                        if gj2 >= 3:
                            pe.wait_ge(dve2_sem, gj2 - 2)
                        for k in range(KT2):
                            if idx2 == 0 and k % GK == 0:
                                pe.wait_ge(w2g_sems[k // GK], 16 * GK * (rep + 1))
                            mm = pe.matmul(po(gj2),
                                           are_t(c, k)[:, tsub * 128:tsub * 128 + 128],
                                           w2_sb[:, k * H + h * HW2:k * H + h * HW2 + HW2],
                                           start=(k == 0), stop=(k == KT2 - 1))
                        mm.then_inc(mm2_sem, 1)

                @block.vector
                def _(dve, rep=rep, po=po, po2=po2, b2bc_sb=b2bc_sb):
                    if h_inner:
                        for g in range(CH * TS):
                            c, tsub = divmod(g, TS)
                            gg = rep * CH * TS + g
                            col = c * TS + tsub
                            if g == 0:
                                dve.wait_ge(b2_sem, 16 * (rep + 1))
                            dve.wait_ge(mm2_sem, gg + 1)
                            for h in range(HT2):
                                gj2 = rep * N2 + g * HT2 + h
                                if gj2 >= 3:
                                    dve.wait_ge(op_sems[gj2 % 3], 16 * (gj2 // 3))
                                dve.tensor_tensor(outs(gj2), po2(gg, h),
                                                  b2bc_sb[:, h * HW2:(h + 1) * HW2],
                                                  mybir.AluOpType.add)
                                dve.tensor_scalar_mul(outs(gj2), outs(gj2),
                                                      rw_sb[:, col:col + 1]).then_inc(dve2_sem, 1)
                        return
                    for idx2 in range(N2):
                        gj2 = rep * N2 + idx2
                        c, r0 = divmod(idx2, PC2)
                        tsub, h = divmod(r0, HT2)
                        col = c * TS + tsub
                        dve.wait_ge(mm2_sem, gj2 + 1)
                        if gj2 >= 3:
                            dve.wait_ge(op_sems[gj2 % 3], 16 * (gj2 // 3))
                        dve.tensor_tensor(outs(gj2), po(gj2),
                                          b2bc_sb[:, h * HW2:(h + 1) * HW2],
                                          mybir.AluOpType.add)
                        dve.tensor_scalar_mul(outs(gj2), outs(gj2),
                                              rw_sb[:, col:col + 1]).then_inc(dve2_sem, 1)

                @block.gpsimd
                def _(gp, rep=rep):
                    if rep > 0:
                        gp.wait_ge(fin_sem, 16 * CCR * rep)  # rs_out re-written below
                    for i in range(RSN):
                        G = rep * N2 + RPG * (i + 1)
                        for s in range(3):
                            v = 16 * _cnt(G, s, 3)
                            if v > 0:
                                gp.wait_ge(op_sems[s], v)
                        gp.collective_compute(
                            "ReduceScatter",
                            mybir.AluOpType.add,
                            ins=[partial[i * RSC:(i + 1) * RSC, :]],
                            outs=[rs_out[i * RSO:(i + 1) * RSO, :]],
                            replica_groups=[list(range(N_CORES))],
                        ).then_inc(cc_sem, 1)
                        if i >= 1:
                            gp.wait_ge(cc_sem, rep * CCR + i)
                            gp.dma_start(out=out_ext[(i - 1) * RSO:i * RSO, :],
                                         in_=rs_out[(i - 1) * RSO:i * RSO, :]).then_inc(fin_sem, 16)
                    i = RSN - 1
                    gp.wait_ge(cc_sem, rep * CCR + RSN)
                    gp.dma_start(out=out_ext[i * RSO:(i + 1) * RSO, :],
                                 in_=rs_out[i * RSO:(i + 1) * RSO, :]).then_inc(fin_sem, 16)
                    if dbg:
                        gp.dma_start(out=dbg_ext[:], in_=partial[:]).then_inc(fin_sem, 16)

    return nc


def _prep_inputs(hidden_states, gate_up_proj, gate_up_proj_bias, down_proj,
                 down_proj_bias, routing_weights):
    """Host-side shard prep: per-core input dicts (core e = expert e)."""
    bf16 = ml_dtypes.bfloat16
    E, H, D2 = gate_up_proj.shape
    D = D2 // 2
    x = np.asarray(hidden_states, np.float32).reshape(-1, hidden_states.shape[-1])
    T = x.shape[0]
    xt = np.ascontiguousarray(x.T).astype(bf16)                 # [H, T]
    rw = np.asarray(routing_weights, np.float32)                # [T, E]
    in_maps = []
    for e in range(E):
        w1 = np.asarray(gate_up_proj[e], np.float32)            # [H, 2D]
        b1 = np.asarray(gate_up_proj_bias[e], np.float32)       # [2D]
        in_maps.append({
            "xt": xt,
            "w1g": np.ascontiguousarray(w1[:, 0::2]).astype(bf16),
            "w1u": np.ascontiguousarray(w1[:, 1::2]).astype(bf16),
            "w2": (np.asarray(down_proj[e], np.float32) / ALPHA).astype(bf16),
            "b1g": np.ascontiguousarray(b1[0::2].reshape(D // 128, 128).T),
            "b1u": np.ascontiguousarray(b1[1::2].reshape(D // 128, 128).T),
            "b2bc": np.ascontiguousarray(np.broadcast_to(
                np.asarray(down_proj_bias[e], np.float32), (128, H))),
            "rw": np.ascontiguousarray(rw[:, e].reshape(T // 128, 128).T),
        })
    return in_maps


def _unshard(outs, T, H, TCH):
    """Reassemble uniform per-256-token ReduceScatter slices: core r's out
    rows [i*RSO, (i+1)*RSO) are global tokens i*RSC + r*RSO + [0, RSO)."""
    RSC = 256
    RSO = RSC // N_CORES
    full = np.empty((T, H), np.float32)
    for r in range(N_CORES):
        o = np.asarray(outs[r], np.float32)
        for i in range(T // RSC):
            full[i * RSC + r * RSO:i * RSC + (r + 1) * RSO] = \
                o[i * RSO:(i + 1) * RSO]
    return full


def kernel(hidden_states, gate_up_proj, gate_up_proj_bias, down_proj,
           down_proj_bias, routing_weights, router_indices=None):
    B, S, H = hidden_states.shape
    T = B * S
    D = down_proj.shape[1]
    TCH = 512
    nc = build_nc(H, D, T, TCH=TCH)
    in_maps = _prep_inputs(hidden_states, gate_up_proj, gate_up_proj_bias,
                           down_proj, down_proj_bias, routing_weights)
    res = run_bass_kernel_spmd(nc, in_maps, core_ids=list(range(N_CORES)))
    full = _unshard([res.results[i]["out"] for i in range(N_CORES)], T, H, TCH)
    return full.reshape(B, S, H).astype(np.float32)



# revision 8
# speedup vs baseline: 1.0962x; 1.0962x over previous
"""Trainium2 Bass kernel: dense MoE (BastileGptOssExperts) via expert parallelism.

Reference math (per token t, hidden h):
    out[t,h] = sum_e rw[t,e] * ( geglu(x @ W1_e + b1_e) @ W2_e + b2_e )[t,h]
with geglu(gate_up): gate = gu[..., 0::2], up = gu[..., 1::2]
    gate_c = min(gate, 7); up_c = clip(up, -7, 7)
    act = (up_c + 1) * gate_c * sigmoid(1.702 * gate_c)

8 cores, expert e on core e. Each core computes its expert over all T
tokens, weights the result by rw[:, e]; a per-token-chunk ReduceScatter
sums over cores and leaves each core with interleaved T/8 token slices;
the host reassembles.

Phase 1: gate_upT[d,t] = sum_k W1[k,d]^T xT[k,t] (W1 resident bf16,
  xT chunked/double-buffered), geglu fused on DVE+ACT:
    DVE: tg = (psum_g + b1g) min 7         [per-partition bias, dual-op]
    ACT: glu' = Silu(1.702 * tg)           [= 1.702 * glu_ref, bf16]
    DVE: tu = (psum_u + b1u) min 7
    ACT: r = Relu(tu + 7)                  [= clip(u,±7) + 7, bf16]
    DVE: act' = (r - 6) * glu'             [= 1.702 * act_ref, bf16]
  act' spilled to DRAM [D, T] bf16. The 1/1.702 is folded into W2 on host.
Phase 2: psum[t,h] = sum_k actT[k,t]^T W2'[k,h]; DVE adds the broadcast
  b2 tile and scales by rw[t] per-partition, DMA'd to partial [T, H] f32.
Uniform per-256-token ReduceScatters (pipelined against phase-2 compute,
with the rs_out->out copy lagging one group) -> each core's out slices.

DMA semaphores are per-buffer-slot so every wait threshold is exact even
if DMA completions reorder across in-flight transfers.

build_nc(reps=N) emits the whole pipeline N times (same buffers; sem
thresholds continue) so on-device time can be measured as a slope.
"""
import sys
sys.path.insert(0, "/opt/trn_rl_repo")

import numpy as np
import ml_dtypes

import concourse.bass as bass
import concourse.mybir as mybir
from concourse.bass_utils import run_bass_kernel_spmd

ALPHA = 1.702
LIMIT = 7.0
BF16 = mybir.dt.bfloat16
F32 = mybir.dt.float32
N_CORES = 8


def _cnt(G, s, m):
    """#{i in [0, G) : i % m == s}"""
    return G // m + (1 if s < G % m else 0)


def build_nc(H, D, T, TCH=512, reps=1, dbg=False, trickle=True, h_inner=True):
    KT = H // 128
    DT = D // 128
    KT2 = D // 128
    CH = T // TCH
    TS = TCH // 128
    HW2 = min(512, H)       # phase-2 matmul moving width (walrus caps at 512)
    HT2 = H // HW2
    N1 = CH * DT
    N2 = CH * TS * HT2
    PC2 = TS * HT2          # phase-2 tiles per chunk
    TCOL = T // 128
    RSC = 1024              # tokens per ReduceScatter (uniform size required!)
    RSN = T // RSC          # collectives per rep
    RPG = RSC // 128 * HT2  # phase-2 out tiles per RS group
    RSO = RSC // N_CORES    # rs_out rows per group
    CCR = RSN
    W2G = 4                 # w2 load k-groups (trickle)
    M2 = CH * TS if h_inner else N2          # mm2_sem increments per rep
    M2C = TS if h_inner else PC2             # ... per chunk

    nc = bass.Bass()

    xt_ext = nc.declare_dram_parameter("xt", [H, T], BF16, isOutput=False)
    w1g_ext = nc.declare_dram_parameter("w1g", [H, D], BF16, isOutput=False)
    w1u_ext = nc.declare_dram_parameter("w1u", [H, D], BF16, isOutput=False)
    w2_ext = nc.declare_dram_parameter("w2", [D, H], BF16, isOutput=False)
    b1g_ext = nc.declare_dram_parameter("b1g", [128, DT], F32, isOutput=False)
    b1u_ext = nc.declare_dram_parameter("b1u", [128, DT], F32, isOutput=False)
    b2bc_ext = nc.declare_dram_parameter("b2bc", [128, H], F32, isOutput=False)
    rw_ext = nc.declare_dram_parameter("rw", [128, TCOL], F32, isOutput=False)
    out_ext = nc.declare_dram_parameter("out", [T // N_CORES, H], BF16, isOutput=True)
    dbg_ext = nc.declare_dram_parameter("dbg", [T, H], BF16, isOutput=True) if dbg else None

    act_sp = nc.dram_tensor("act_sp", [D, T], BF16)
    partial = nc.dram_tensor("partial", [T, H], BF16)
    rs_out = nc.dram_tensor("rs_out", [T // N_CORES, H], BF16)
    RSCH = TCH // N_CORES   # rs_out rows per chunk

    from contextlib import ExitStack
    with ExitStack() as ctx:
        ec = ctx.enter_context
        b1g_sb = ec(nc.sbuf_tensor("b1g_sb", [128, DT], F32))
        b1u_sb = ec(nc.sbuf_tensor("b1u_sb", [128, DT], F32))
        rw_sb = ec(nc.sbuf_tensor("rw_sb", [128, TCOL], F32))
        lim_sb = ec(nc.sbuf_tensor("lim_sb", [128, 1], F32))
        tg_sb = ec(nc.sbuf_tensor("tg_sb", [128, 2 * 512], BF16))
        tu_sb = ec(nc.sbuf_tensor("tu_sb", [128, 2 * 512], BF16))
        glu_sb = ec(nc.sbuf_tensor("glu_sb", [128, 2 * 512], BF16))
        r_sb = ec(nc.sbuf_tensor("r_sb", [128, 2 * 512], F32))
        acto_sb = ec(nc.sbuf_tensor("acto_sb", [128, 3 * 512], BF16))
        # compute-side semaphores (single-engine in-order increments)
        mm_sem = ec(nc.semaphore("mm_sem"))
        mm2_sem = ec(nc.semaphore("mm2_sem"))
        psum_free_sem = ec(nc.semaphore("psum_free_sem"))
        v1_sem = ec(nc.semaphore("v1_sem"))
        a_sem = ec(nc.semaphore("a_sem"))
        act_ready_sem = ec(nc.semaphore("act_ready_sem"))
        dve2_sem = ec(nc.semaphore("dve2_sem"))
        cc_sem = ec(nc.semaphore("cc_sem"))
        fin_sem = ec(nc.semaphore("fin_sem"))
        # DMA semaphores: per buffer slot so thresholds are exact
        dma_c_sem = ec(nc.semaphore("dma_c_sem"))
        w_sem = ec(nc.semaphore("w_sem"))
        w2g_sems = [ec(nc.semaphore(f"w2g_sem{i}")) for i in range(W2G)]
        wg_sems = [ec(nc.semaphore(f"wg_sem{i}")) for i in range(4)]
        xt_sems = [ec(nc.semaphore(f"xt_sem{i}")) for i in range(4)]
        sp_sems = [ec(nc.semaphore(f"sp_sem{i}")) for i in range(3)]
        rel_sems = [ec(nc.semaphore(f"rel_sem{i}")) for i in range(2)]
        op_sems = [ec(nc.semaphore(f"op_sem{i}")) for i in range(3)]
        b2_sem = ec(nc.semaphore("b2_sem"))
        block = ec(nc.Block())

        def tg(i):
            return tg_sb[:, (i % 2) * 512:(i % 2) * 512 + 512]

        def tu(i):
            return tu_sb[:, (i % 2) * 512:(i % 2) * 512 + 512]

        def glu(i):
            return glu_sb[:, (i % 2) * 512:(i % 2) * 512 + 512]

        def rr(i):
            return r_sb[:, (i % 2) * 512:(i % 2) * 512 + 512]

        def acto(i):
            return acto_sb[:, (i % 3) * 512:(i % 3) * 512 + 512]

        outs_holder = {}

        def outs(i):
            return outs_holder["sb"][:, (i % 3) * HW2:(i % 3) * HW2 + HW2]

        for rep in range(reps):
            # ======================= PHASE 1 =======================
            with (
                nc.sbuf_tensor(f"w1g_sb{rep}", [128, KT * D], BF16) as w1g_sb,
                nc.sbuf_tensor(f"w1u_sb{rep}", [128, KT * D], BF16) as w1u_sb,
                nc.sbuf_tensor(f"xt_sb{rep}", [128, 4 * KT * TCH], BF16) as xt_sb,
                nc.psum_tensor(f"psg{rep}", [128, 4 * 512], F32) as psg,
                nc.psum_tensor(f"psu{rep}", [128, 4 * 512], F32) as psu,
            ):
                def xt_t(c, k):
                    s = (c % 4) * KT * TCH
                    return xt_sb[:, s + k * TCH:s + k * TCH + TCH]

                def pg(pidx):  # slot by (q parity, pair member)
                    s = (((pidx // 2) % 2) * 2 + (pidx % 2)) * 512
                    return psg[:, s:s + 512]

                def pu(pidx):
                    s = (((pidx // 2) % 2) * 2 + (pidx % 2)) * 512
                    return psu[:, s:s + 512]

                @block.sync
                def _(sp, rep=rep, xt_t=xt_t):
                    if rep == 0:
                        sp.dma_start(out=b1g_sb[:], in_=b1g_ext[:]).then_inc(dma_c_sem, 16)
                        sp.dma_start(out=b1u_sb[:], in_=b1u_ext[:]).then_inc(dma_c_sem, 16)
                        sp.dma_start(out=rw_sb[:], in_=rw_ext[:]).then_inc(dma_c_sem, 16)
                    else:
                        # w1/xt regions were reused by phase 2 of rep-1
                        sp.wait_ge(mm2_sem, rep * M2)
                    GK1 = max(1, KT // 4)
                    for c in range(min(4, CH)):
                        gc = rep * CH + c
                        if rep == 0 and c == 0:
                            # interleave w1 + xt0 + xt1 loads in k-groups so PE
                            # can start pair 0 after the first group lands
                            for k in range(KT):
                                g = k // GK1
                                sp.dma_start(out=w1g_sb[:, k * D:(k + 1) * D],
                                             in_=w1g_ext[k * 128:(k + 1) * 128, :]).then_inc(wg_sems[g], 16)
                                sp.dma_start(out=w1u_sb[:, k * D:(k + 1) * D],
                                             in_=w1u_ext[k * 128:(k + 1) * 128, :]).then_inc(wg_sems[g], 16)
                                sp.dma_start(out=xt_t(0, k),
                                             in_=xt_ext[k * 128:(k + 1) * 128, 0:TCH]).then_inc(wg_sems[g], 16)
                                if CH > 1:
                                    sp.dma_start(out=xt_t(1, k),
                                                 in_=xt_ext[k * 128:(k + 1) * 128,
                                                            TCH:2 * TCH]).then_inc(wg_sems[g], 16)
                            continue
                        if rep == 0 and c == 1:
                            continue  # loaded with pair 0 above
                        for k in range(KT):
                            sp.dma_start(out=xt_t(c, k),
                                         in_=xt_ext[k * 128:(k + 1) * 128,
                                                    c * TCH:(c + 1) * TCH]).then_inc(xt_sems[gc % 4], 16)
                        if c == 0:
                            for k in range(KT):
                                sp.dma_start(out=w1g_sb[:, k * D:(k + 1) * D],
                                             in_=w1g_ext[k * 128:(k + 1) * 128, :]).then_inc(w_sem, 16)
                                sp.dma_start(out=w1u_sb[:, k * D:(k + 1) * D],
                                             in_=w1u_ext[k * 128:(k + 1) * 128, :]).then_inc(w_sem, 16)
                    for pidx in range(N1):
                        gj = rep * N1 + pidx
                        q, which = divmod(pidx, 2)
                        p, d = divmod(q, DT)
                        c = 2 * p + which
                        sp.wait_ge(act_ready_sem, gj + 1)
                        sp.dma_start(out=act_sp[d * 128:(d + 1) * 128, c * TCH:(c + 1) * TCH],
                                     in_=acto(gj)).then_inc(sp_sems[gj % 3], 16)
                        if pidx == 2 * DT * (p + 1) - 1 and 2 * (p + 2) < CH:
                            sp.wait_ge(mm_sem, 2 * rep * N1 + 4 * DT * (p + 1))
                            for cc in (2 * (p + 2), 2 * (p + 2) + 1):
                                gc2 = rep * CH + cc
                                for k in range(KT):
                                    sp.dma_start(out=xt_t(cc, k),
                                                 in_=xt_ext[k * 128:(k + 1) * 128,
                                                            cc * TCH:(cc + 1) * TCH]).then_inc(xt_sems[gc2 % 4], 16)

                @block.tensor
                def _(pe, rep=rep, xt_t=xt_t, pg=pg, pu=pu):
                    GK1 = max(1, KT // 4)
                    NP = CH // 2          # chunk pairs
                    for q in range(NP * DT):
                        p, d = divmod(q, DT)
                        gq = rep * NP * DT + q
                        c0, c1 = 2 * p, 2 * p + 1
                        gc0, gc1 = rep * CH + c0, rep * CH + c1
                        pA = rep * N1 + 2 * q      # pidx of pair members
                        if d == 0 and gc0 > 0:
                            for gcx in (gc0, gc1):
                                n = (gcx - 2) // 4 + 1
                                pe.wait_ge(xt_sems[gcx % 4], 16 * KT * n)
                        if q == 0 and rep > 0:
                            pe.wait_ge(w_sem, 16 * 2 * KT * rep)
                            pe.wait_ge(dve2_sem, rep * N2)
                        if gq >= 2:
                            pe.wait_ge(psum_free_sem, 2 * gq - 2)
                        for k in range(KT):
                            if gc0 == 0 and k % GK1 == 0:
                                pe.wait_ge(wg_sems[k // GK1], 16 * 4 * GK1)
                            mmA = pe.matmul(pg(pA), w1g_sb[:, k * D + d * 128:k * D + d * 128 + 128],
                                            xt_t(c0, k), start=(k == 0), stop=(k == KT - 1))
                            mmB = pe.matmul(pg(pA + 1), w1g_sb[:, k * D + d * 128:k * D + d * 128 + 128],
                                            xt_t(c1, k), start=(k == 0), stop=(k == KT - 1))
                        mmA.then_inc(mm_sem, 1)
                        mmB.then_inc(mm_sem, 1)
                        for k in range(KT):
                            mmA = pe.matmul(pu(pA), w1u_sb[:, k * D + d * 128:k * D + d * 128 + 128],
                                            xt_t(c0, k), start=(k == 0), stop=(k == KT - 1))
                            mmB = pe.matmul(pu(pA + 1), w1u_sb[:, k * D + d * 128:k * D + d * 128 + 128],
                                            xt_t(c1, k), start=(k == 0), stop=(k == KT - 1))
                        mmA.then_inc(mm_sem, 1)
                        mmB.then_inc(mm_sem, 1)

                @block.vector
                def _(dve, rep=rep, pg=pg, pu=pu):
                    if rep == 0:
                        dve.memset(lim_sb[:], LIMIT)
                        dve.wait_ge(dma_c_sem, 48)
                    for idx in range(N1):
                        gj = rep * N1 + idx
                        if gj >= 2:
                            dve.wait_ge(a_sem, gj - 1)
                        q, which = divmod(idx, 2)
                        d = q % DT
                        mmbase = 2 * rep * N1 + 4 * q
                        dve.wait_ge(mm_sem, mmbase + 1 + which)
                        dve.tensor_scalar(tg(gj), pg(gj), b1g_sb[:, d:d + 1], LIMIT,
                                          mybir.AluOpType.add, mybir.AluOpType.min).then_inc(v1_sem, 1)
                        dve.wait_ge(mm_sem, mmbase + 3 + which)
                        dve.tensor_scalar(tu(gj), pu(gj), b1u_sb[:, d:d + 1], LIMIT,
                                          mybir.AluOpType.add, mybir.AluOpType.min).then_inc(psum_free_sem, 1)
                        j = gj - 1
                        if j >= rep * N1:
                            dve.wait_ge(a_sem, j + 1)
                            if j >= 3:
                                dve.wait_ge(sp_sems[j % 3], 16 * (j // 3))
                            dve.scalar_tensor_tensor(acto(j), rr(j), 6.0, glu(j),
                                                     mybir.AluOpType.subtract,
                                                     mybir.AluOpType.mult).then_inc(act_ready_sem, 1)
                    j = rep * N1 + N1 - 1
                    dve.wait_ge(a_sem, j + 1)
                    if j >= 3:
                        dve.wait_ge(sp_sems[j % 3], 16 * (j // 3))
                    dve.scalar_tensor_tensor(acto(j), rr(j), 6.0, glu(j),
                                             mybir.AluOpType.subtract,
                                             mybir.AluOpType.mult).then_inc(act_ready_sem, 1)

                @block.scalar
                def _(act, rep=rep):
                    for idx in range(N1):
                        gj = rep * N1 + idx
                        if gj >= 2:
                            act.wait_ge(act_ready_sem, gj - 1)
                        act.wait_ge(v1_sem, gj + 1)
                        act.activation(glu(gj), tg(gj),
                                       mybir.ActivationFunctionType.Silu, scale=ALPHA)
                        act.wait_ge(psum_free_sem, gj + 1)
                        act.activation(rr(gj), tu(gj),
                                       mybir.ActivationFunctionType.Relu,
                                       bias=lim_sb[:, 0:1]).then_inc(a_sem, 1)

            # ======================= PHASE 2 =======================
            with (
                nc.sbuf_tensor(f"w2_sb{rep}", [128, KT2 * H], BF16) as w2_sb,
                nc.sbuf_tensor(f"actre_sb{rep}", [128, 2 * KT2 * TCH], BF16) as actre_sb,
                nc.sbuf_tensor(f"b2bc_sb{rep}", [128, H], F32) as b2bc_sb,
                nc.sbuf_tensor(f"outs_sb{rep}", [128, 3 * HW2], BF16) as outs_sb,
                nc.psum_tensor(f"pso{rep}", [128, (8 if h_inner else 3) * HW2], F32) as pso,
            ):
                def are_t(c, k):
                    s = (c % 2) * KT2 * TCH
                    return actre_sb[:, s + k * TCH:s + k * TCH + TCH]

                def po(i):
                    return pso[:, (i % 3) * HW2:(i % 3) * HW2 + HW2]

                def po2(g, h):
                    s = ((g % 2) * HT2 + h) * HW2
                    return pso[:, s:s + HW2]

                outs_holder["sb"] = outs_sb

                @block.sync
                def _(sp, rep=rep, are_t=are_t, b2bc_sb=b2bc_sb):
                    sp.wait_ge(mm_sem, 2 * N1 * (rep + 1))   # w1/xt regions free
                    if rep > 0:
                        # outs/b2bc overlay region also held prev rep's out tiles
                        for s in range(3):
                            v = 16 * _cnt(rep * N2, s, 3)
                            if v > 0:
                                sp.wait_ge(op_sems[s], v)
                    sp.dma_start(out=b2bc_sb[:], in_=b2bc_ext[:]).then_inc(b2_sem, 16)
                    GK = KT2 // W2G
                    # reload chunk 0 first (needed as soon as w2 group 0 lands)
                    G = rep * N1 + 2 * DT
                    for s in range(3):
                        v = 16 * _cnt(G, s, 3)
                        if v > 0:
                            sp.wait_ge(sp_sems[s], v)
                    for k in range(KT2):
                        sp.dma_start(out=are_t(0, k),
                                     in_=act_sp[k * 128:(k + 1) * 128, 0:TCH]).then_inc(
                                         rel_sems[(rep * CH) % 2], 16)
                    for k in range(KT2):
                        sp.dma_start(out=w2_sb[:, k * H:(k + 1) * H],
                                     in_=w2_ext[k * 128:(k + 1) * 128, :]).then_inc(
                                         w2g_sems[k // GK], 16)
                    if CH > 1:
                        gc = rep * CH + 1
                        G = rep * N1 + 2 * DT
                        for s in range(3):
                            v = 16 * _cnt(G, s, 3)
                            if v > 0:
                                sp.wait_ge(sp_sems[s], v)
                        for k in range(KT2):
                            sp.dma_start(out=are_t(1, k),
                                         in_=act_sp[k * 128:(k + 1) * 128,
                                                    TCH:2 * TCH]).then_inc(rel_sems[gc % 2], 16)
                    for idx2 in range(N2):
                        gj2 = rep * N2 + idx2
                        c, r0 = divmod(idx2, PC2)
                        tsub, h = divmod(r0, HT2)
                        if rep > 0 and r0 == 0:
                            # partial chunk c is re-written; previous rep's RS
                            # groups covering it must have consumed it
                            sp.wait_ge(cc_sem, (rep - 1) * CCR
                                       + ((c + 1) * TCH + RSC - 1) // RSC)
                        sp.wait_ge(dve2_sem, gj2 + 1)
                        sp.dma_start(out=partial[c * TCH + tsub * 128:c * TCH + tsub * 128 + 128,
                                                 h * HW2:(h + 1) * HW2],
                                     in_=outs(gj2)).then_inc(op_sems[gj2 % 3], 16)
                        if r0 == PC2 - 1 and (c + 2) < CH:
                            gc2 = rep * CH + c + 2
                            sp.wait_ge(mm2_sem, rep * M2 + M2C * (c + 1))
                            G = rep * N1 + 2 * DT * ((c + 2) // 2 + 1)
                            for s in range(3):
                                v = 16 * _cnt(G, s, 3)
                                if v > 0:
                                    sp.wait_ge(sp_sems[s], v)
                            for k in range(KT2):
                                sp.dma_start(out=are_t(c + 2, k),
                                             in_=act_sp[k * 128:(k + 1) * 128,
                                                        (c + 2) * TCH:(c + 3) * TCH]).then_inc(rel_sems[gc2 % 2], 16)

                @block.tensor
                def _(pe, rep=rep, are_t=are_t, po=po, po2=po2):
                    GK = KT2 // W2G
                    if h_inner:
                        for g in range(CH * TS):
                            c, tsub = divmod(g, TS)
                            gg = rep * CH * TS + g
                            gc = rep * CH + c
                            if g == 0:
                                pe.wait_ge(psum_free_sem, N1 * (rep + 1))
                            if tsub == 0:
                                pe.wait_ge(rel_sems[gc % 2], 16 * KT2 * (gc // 2 + 1))
                            if gg >= 2:
                                # bank-set gg%2 reused from group gg-2: its
                                # HT2 tiles must be consumed by DVE4
                                pe.wait_ge(dve2_sem, HT2 * (gg - 1))
                            for k in range(KT2):
                                if g == 0 and k % GK == 0:
                                    pe.wait_ge(w2g_sems[k // GK], 16 * GK * (rep + 1))
                                for h in range(HT2):
                                    mm = pe.matmul(po2(gg, h),
                                                   are_t(c, k)[:, tsub * 128:tsub * 128 + 128],
                                                   w2_sb[:, k * H + h * HW2:k * H + h * HW2 + HW2],
                                                   start=(k == 0), stop=(k == KT2 - 1))
                            mm.then_inc(mm2_sem, 1)
                        return
                    for idx2 in range(N2):
                        gj2 = rep * N2 + idx2
                        c, r0 = divmod(idx2, PC2)
                        tsub, h = divmod(r0, HT2)
                        gc = rep * CH + c
                        GK = KT2 // W2G
                        if idx2 == 0:
                            pe.wait_ge(psum_free_sem, N1 * (rep + 1))
                        if r0 == 0:
                            pe.wait_ge(rel_sems[gc % 2], 16 * KT2 * (gc // 2 + 1))
                        if gj2 >= 3:
                            pe.wait_ge(dve2_sem, gj2 - 2)
                        for k in range(KT2):
                            if idx2 == 0 and k % GK == 0:
                                pe.wait_ge(w2g_sems[k // GK], 16 * GK * (rep + 1))
                            mm = pe.matmul(po(gj2),
                                           are_t(c, k)[:, tsub * 128:tsub * 128 + 128],
                                           w2_sb[:, k * H + h * HW2:k * H + h * HW2 + HW2],
                                           start=(k == 0), stop=(k == KT2 - 1))
                        mm.then_inc(mm2_sem, 1)

                @block.vector
                def _(dve, rep=rep, po=po, po2=po2, b2bc_sb=b2bc_sb):
                    if h_inner:
                        for g in range(CH * TS):
                            c, tsub = divmod(g, TS)
                            gg = rep * CH * TS + g
                            col = c * TS + tsub
                            if g == 0:
                                dve.wait_ge(b2_sem, 16 * (rep + 1))
                            dve.wait_ge(mm2_sem, gg + 1)
                            for h in range(HT2):
                                gj2 = rep * N2 + g * HT2 + h
                                if gj2 >= 3:
                                    dve.wait_ge(op_sems[gj2 % 3], 16 * (gj2 // 3))
                                dve.tensor_tensor(outs(gj2), po2(gg, h),
                                                  b2bc_sb[:, h * HW2:(h + 1) * HW2],
                                                  mybir.AluOpType.add)
                                dve.tensor_scalar_mul(outs(gj2), outs(gj2),
                                                      rw_sb[:, col:col + 1]).then_inc(dve2_sem, 1)
                        return
                    for idx2 in range(N2):
                        gj2 = rep * N2 + idx2
                        c, r0 = divmod(idx2, PC2)
                        tsub, h = divmod(r0, HT2)
                        col = c * TS + tsub
                        dve.wait_ge(mm2_sem, gj2 + 1)
                        if gj2 >= 3:
                            dve.wait_ge(op_sems[gj2 % 3], 16 * (gj2 // 3))
                        dve.tensor_tensor(outs(gj2), po(gj2),
                                          b2bc_sb[:, h * HW2:(h + 1) * HW2],
                                          mybir.AluOpType.add)
                        dve.tensor_scalar_mul(outs(gj2), outs(gj2),
                                              rw_sb[:, col:col + 1]).then_inc(dve2_sem, 1)

                @block.gpsimd
                def _(gp, rep=rep):
                    if rep > 0:
                        gp.wait_ge(fin_sem, 16 * CCR * rep)  # rs_out re-written below
                    for i in range(RSN):
                        G = rep * N2 + RPG * (i + 1)
                        for s in range(3):
                            v = 16 * _cnt(G, s, 3)
                            if v > 0:
                                gp.wait_ge(op_sems[s], v)
                        gp.collective_compute(
                            "ReduceScatter",
                            mybir.AluOpType.add,
                            ins=[partial[i * RSC:(i + 1) * RSC, :]],
                            outs=[rs_out[i * RSO:(i + 1) * RSO, :]],
                            replica_groups=[list(range(N_CORES))],
                        ).then_inc(cc_sem, 1)
                        if i >= 1:
                            gp.wait_ge(cc_sem, rep * CCR + i)
                            gp.dma_start(out=out_ext[(i - 1) * RSO:i * RSO, :],
                                         in_=rs_out[(i - 1) * RSO:i * RSO, :]).then_inc(fin_sem, 16)
                    i = RSN - 1
                    gp.wait_ge(cc_sem, rep * CCR + RSN)
                    gp.dma_start(out=out_ext[i * RSO:(i + 1) * RSO, :],
                                 in_=rs_out[i * RSO:(i + 1) * RSO, :]).then_inc(fin_sem, 16)
                    if dbg:
                        gp.dma_start(out=dbg_ext[:], in_=partial[:]).then_inc(fin_sem, 16)

    return nc


def _prep_inputs(hidden_states, gate_up_proj, gate_up_proj_bias, down_proj,
                 down_proj_bias, routing_weights):
    """Host-side shard prep: per-core input dicts (core e = expert e)."""
    bf16 = ml_dtypes.bfloat16
    E, H, D2 = gate_up_proj.shape
    D = D2 // 2
    x = np.asarray(hidden_states, np.float32).reshape(-1, hidden_states.shape[-1])
    T = x.shape[0]
    xt = np.ascontiguousarray(x.T).astype(bf16)                 # [H, T]
    rw = np.asarray(routing_weights, np.float32)                # [T, E]
    in_maps = []
    for e in range(E):
        w1 = np.asarray(gate_up_proj[e], np.float32)            # [H, 2D]
        b1 = np.asarray(gate_up_proj_bias[e], np.float32)       # [2D]
        in_maps.append({
            "xt": xt,
            "w1g": np.ascontiguousarray(w1[:, 0::2]).astype(bf16),
            "w1u": np.ascontiguousarray(w1[:, 1::2]).astype(bf16),
            "w2": (np.asarray(down_proj[e], np.float32) / ALPHA).astype(bf16),
            "b1g": np.ascontiguousarray(b1[0::2].reshape(D // 128, 128).T),
            "b1u": np.ascontiguousarray(b1[1::2].reshape(D // 128, 128).T),
            "b2bc": np.ascontiguousarray(np.broadcast_to(
                np.asarray(down_proj_bias[e], np.float32), (128, H))),
            "rw": np.ascontiguousarray(rw[:, e].reshape(T // 128, 128).T),
        })
    return in_maps


def _unshard(outs, T, H, TCH):
    """Reassemble uniform per-RSC-token ReduceScatter slices: core r's out
    rows [i*RSO, (i+1)*RSO) are global tokens i*RSC + r*RSO + [0, RSO)."""
    RSC = 1024
    RSO = RSC // N_CORES
    full = np.empty((T, H), np.float32)
    for r in range(N_CORES):
        o = np.asarray(outs[r], np.float32)
        for i in range(T // RSC):
            full[i * RSC + r * RSO:i * RSC + (r + 1) * RSO] = \
                o[i * RSO:(i + 1) * RSO]
    return full


def kernel(hidden_states, gate_up_proj, gate_up_proj_bias, down_proj,
           down_proj_bias, routing_weights, router_indices=None):
    B, S, H = hidden_states.shape
    T = B * S
    D = down_proj.shape[1]
    TCH = 512
    nc = build_nc(H, D, T, TCH=TCH)
    in_maps = _prep_inputs(hidden_states, gate_up_proj, gate_up_proj_bias,
                           down_proj, down_proj_bias, routing_weights)
    res = run_bass_kernel_spmd(nc, in_maps, core_ids=list(range(N_CORES)))
    full = _unshard([res.results[i]["out"] for i in range(N_CORES)], T, H, TCH)
    return full.reshape(B, S, H).astype(np.float32)

